# revision 65
# baseline (speedup 1.0000x reference)
"""Trainium2 Bass kernel for a single-layer MHA + FFN transformer block.

Reference computation (for x: [1, 4096, 768], 12 heads, dff=3072):
    qkv = (x @ w_qkv + b_qkv)  -> q, k, v       [t, 768] each
    scores = q k^T / sqrt(768) ; wei = softmax(scores)
    attn = wei @ v  (concat heads)              [t, 768]
    h = gelu(attn @ w_ff1 + b_ff1)              [t, 3072]
    out = h @ w_ff2 + b_ff2                     [t, 768]

Sharding: sequence-parallel over the 4096 tokens across 8 NeuronCores
(512 rows each). Every core computes q/k/v for its own rows, the k/v
blocks are exchanged with one bf16 AllGather, then each core runs full
attention for its 512 query rows over all 4096 keys plus the whole FFN
for its rows. Outputs are concatenated on the host.

Key structure (v2):
 - Phase A: the diagonal attention block (this core's own 4 key chunks)
   runs ENTIRELY inside the AllGather window using only local tiles; the
   per-pair partial numerators+denominators park in SBUF (bf16) and later
   seed the phase-B PSUM accumulators via an identity matmul.
 - The AllGather slot is staged partition-major so each REMOTE rank's
   whole k+v block reads back with a single dynamic-offset DMA whose DRAM
   address is computed from the partition id: the local rank is skipped
   and phase B only processes the 28 remote key chunks per head pair.
 - Softmax normalization: one merged [33,TL] reciprocal per pair (rows
   0/32 carry the two heads' denominators) -> one K=2 sel33 broadcast
   matmul -> one multiply, emitted mid-pair so boundaries stay clean.
 - FFN2 is computed transposed (outT[d,t] = sum_ff w2[ff,d] h[t,ff]) as
   six N=512 PSUM accumulations interleaved with FFN1/gelu - no separate
   second sweep; bias is a per-partition add; the host transposes y back.

The v tiles carry a per-head ones column (width 65*12=780) so softmax
denominators ride along in the wei@v matmuls. Softmax skips
max-subtraction because the logits here are bounded by ~0.6.
"""

import json as _json
import math

import numpy as np
import ml_dtypes

import concourse.bass as bass
import concourse.mybir as mybir
import concourse.tile as tile
from concourse.bass_utils import run_bass_kernel_spmd

# ---------------------------------------------------------------------------
# Workaround: the pinned walrus build only supports ONE embedded semaphore
# wait per instruction, but Tile's sem assigner attaches several. Split the
# excess onto standalone EventSemaphore instructions (pure waits) inserted
# just before the over-subscribed instruction (same engine => same program
# order, identical semantics).
# ---------------------------------------------------------------------------
_MAX_WAITS = 1
_ctr = [0]
if not getattr(bass.Bass, "_multiwait_patched", False):
    _orig_to_json_bytes = bass.Bass.to_json_bytes

    def _split_multiwait_json_bytes(self):
        bir = _json.loads(_orig_to_json_bytes(self))
        for f in bir["functions"]:
            for b in f["blocks"]:
                new_insts = []
                for inst in b["instructions"]:
                    si = inst.get("sync_info")
                    waits = si.get("on_wait", []) if si else []
                    if len(waits) > _MAX_WAITS:
                        excess, keep = waits[:-_MAX_WAITS], waits[-_MAX_WAITS:]
                        for k in range(0, len(excess), _MAX_WAITS):
                            _ctr[0] += 1
                            new_insts.append({
                                "debug": inst.get("debug", 0),
                                "engine": inst["engine"],
                                "ins": [], "outs": [],
                                "name": "I-waitsplit-%d" % _ctr[0],
                                "opcode": "EventSemaphore",
                                "sync_info": {"on_update": [],
                                              "on_wait": excess[k:k + _MAX_WAITS]},
                            })
                        si["on_wait"] = keep
                    new_insts.append(inst)
                b["instructions"] = new_insts
        return _json.dumps(bir).encode()

    bass.Bass.to_json_bytes = _split_multiwait_json_bytes
    bass.Bass._multiwait_patched = True

F32 = mybir.dt.float32
F32R = mybir.dt.float32r
BF16 = mybir.dt.bfloat16
AFT = mybir.ActivationFunctionType

R = 8          # cores
T = 4096       # sequence length
TL = T // R    # rows per core (512)
D = 768
H = 12
HD = D // H    # 64
DFF = 4 * D    # 3072
P = 128
NDT = D // P   # 6 d-tiles
NTT = TL // P  # 4 local t-tiles
NFT = DFF // P  # 24 dff tiles
NCH = T // P   # 32 global key chunks
SCALE = 1.0 / math.sqrt(D)
VW = H * (HD + 1)      # 780: v tile width with a ones column per head
V_OFF = D * TL         # v region offset inside a rank slot
SLOT = D * TL + TL * VW  # 792576: per-rank AllGather slot

_NC_CACHE = {}


def _build_nc():
    nc = bass.Bass(num_devices=R)
    # host-prepped per-core inputs
    xTb = nc.declare_dram_parameter("xTb", [P, NDT, TL], BF16, isOutput=False)
    # host-prepped common weights (k and q split so the k-path DMA can be
    # prioritized: k/v projections gate the AllGather trigger)
    wkc = nc.declare_dram_parameter("wkc", [P, NDT, D], BF16, isOutput=False)
    wqc = nc.declare_dram_parameter("wqc", [P, NDT, D], BF16, isOutput=False)
    wv = nc.declare_dram_parameter("wv", [P, NDT, D], BF16, isOutput=False)
    w1h = nc.declare_dram_parameter("w1h", [P, NDT, DFF], BF16, isOutput=False)
    w2b = nc.declare_dram_parameter("w2b", [P, NFT, D], BF16, isOutput=False)
    b_qkv = nc.declare_dram_parameter("b_qkv", [3 * D], F32, isOutput=False)
    b_ff1 = nc.declare_dram_parameter("b_ff1", [DFF], F32, isOutput=False)
    b_ff2 = nc.declare_dram_parameter("b_ff2", [D], F32, isOutput=False)
    # output is stored TRANSPOSED ([D, TL]); the host transposes back. This
    # lets FFN2 run as clean N=512 accumulations over dff with no tail sweep.
    y = nc.declare_dram_parameter("y", [D, TL], F32, isOutput=True)

    from contextlib import ExitStack

    with tile.TileContext(nc) as tc, ExitStack() as top:
        const = top.enter_context(tc.tile_pool(name="const", bufs=1))
        dramp = top.enter_context(tc.tile_pool(name="dramp", bufs=1, space="DRAM"))
        persist = top.enter_context(tc.tile_pool(name="persist", bufs=1))

        ones_dram = nc.inline_tensor(np.ones((1, P), np.float32), name="ones_const")
        ones_row = const.tile([1, P], F32R, name="ones_row")
        nc.sync.dma_start(ones_row[:], ones_dram.ap().bitcast(F32R))
        # head-pair selection for the denominator broadcast: row 0 -> cols
        # 0..63 (even head), row 32 -> cols 64..127 (odd head). Rows 0/32
        # because engine partition bases must be 32-aligned; the unused rows
        # are zero so garbage rec values there cannot leak through the matmul.
        sel_np = np.zeros((33, P), np.float32)
        sel_np[0, 0:HD] = 1.0
        sel_np[32, HD:P] = 1.0
        sel_dram = nc.inline_tensor(sel_np, name="sel33_const")
        sel33 = const.tile([33, P], F32R, name="sel33")
        nc.sync.dma_start(sel33[:], sel_dram.ap().bitcast(F32R))

        bq_sb = const.tile([P, 2 * NDT], F32, name="bq_sb")
        nc.gpsimd.dma_start(
            bq_sb[:], b_qkv.ap()[0:2 * D].rearrange("(o p) -> p o", p=P))
        bv_sb = const.tile([1, D], F32R, name="bv_sb")
        nc.gpsimd.dma_start(bv_sb[:], b_qkv.ap()[None, 2 * D:3 * D].bitcast(F32R))
        b1_sb = const.tile([P, NFT], F32, name="b1_sb")
        nc.gpsimd.dma_start(b1_sb[:], b_ff1.ap().rearrange("(o p) -> p o", p=P))
        b2t_sb = const.tile([P, NDT], F32, name="b2t_sb")
        nc.gpsimd.dma_start(b2t_sb[:], b_ff2.ap().rearrange("(o p) -> p o", p=P))

        # identity for seeding phase-B accumulators from phase-A partials
        i65_dram = nc.inline_tensor(
            np.eye(HD + 1, dtype=np.float32), name="i65_const")
        i65 = const.tile([HD + 1, HD + 1], BF16, name="i65")
        nc.gpsimd.dma_start(i65[:], i65_dram.ap())

        # preload the exp activation table while phase 1 is DMA-bound
        warmup = const.tile([P, 1], F32, name="warmup")
        nc.scalar.activation(warmup[:], bq_sb[:, 0:1], AFT.Exp)

        # single AllGather. k/v are staged PARTITION-MAJOR inside each
        # rank's slot (k: [128, 6*512], v: [128, 4*780]) so each remote
        # rank's whole k (or v) block reads back as ONE contiguous-per-
        # partition 2D DMA whose DRAM offset is a runtime register
        # (rank-dependent), skipping the local rank entirely.
        ag_in = dramp.tile([SLOT], BF16, name="ag_in")
        ag_out = dramp.tile([R * SLOT], BF16, addr_space="Shared",
                            name="ag_out")

        attnT = [persist.tile([P, TL], BF16, name=f"attnT{i}") for i in range(NDT)]
        w1sb = persist.tile([P, NDT, DFF], BF16, name="w1sb")
        w2sb = persist.tile([P, NFT, D], BF16, name="w2sb")

        NRC = NCH - NTT  # 28 remote chunks

        kv_scope = top.enter_context(ExitStack())
        kvp = kv_scope.enter_context(tc.tile_pool(name="kvp", bufs=1))
        qT = [kvp.tile([P, TL], BF16, name=f"qT{i}") for i in range(NDT)]
        # phase-A partial numerators+denominators (row 64), one per head
        locn = [[kvp.tile([HD + 1, TL], BF16, name=f"loc{p}_{s}")
                 for s in range(2)] for p in range(H // 2)]
        # local k/v live only through phase A; their space is then reused
        # by the remote-rank readback tiles (allocated at readback time)
        akv_scope = top.enter_context(ExitStack())
        akvp = akv_scope.enter_context(tc.tile_pool(name="akvp", bufs=1))
        kT_loc = [akvp.tile([P, TL], BF16, name=f"kTl{i}") for i in range(NDT)]
        v_loc = [akvp.tile([P, VW], BF16, name=f"vl{t}") for t in range(NTT)]

        # ------------------------------------------------------------------
        # Phase 1: QKV projections straight from host-transposed x
        # ------------------------------------------------------------------
        with ExitStack() as ph1:
            xp = ph1.enter_context(tc.tile_pool(name="xp", bufs=1))
            psQ = ph1.enter_context(tc.tile_pool(name="psQ", bufs=2, space="PSUM"))
            psV = ph1.enter_context(tc.tile_pool(name="psV", bufs=2, space="PSUM"))

            xtb_sb = xp.tile([P, NDT, TL], BF16, name="xtb")
            nc.sync.dma_start(xtb_sb[:], xTb.ap())
            wk_sb = xp.tile([P, NDT, D], BF16, name="wk_sb")
            nc.sync.dma_start(wk_sb[:], wkc.ap())
            wv_sb = xp.tile([P, NDT, D], BF16, name="wv_sb")
            nc.scalar.dma_start(wv_sb[:], wv.ap())
            wq_sb = wk_sb  # wq reuses wk's buffer once the k proj is done

            def proj_jt(w_sb, jt, bcol, out_ap):
                """qkv^T tile for channel block jt (0..5) of weight w_sb."""
                ps = psQ.tile([P, TL], F32, tag="psq", name="psq")
                for d_ in range(NDT):
                    nc.tensor.matmul(ps[:], w_sb[:, d_, P * jt:P * (jt + 1)],
                                     xtb_sb[:, d_, :],
                                     start=(d_ == 0), stop=(d_ == NDT - 1))
                nc.vector.tensor_scalar_add(out_ap, ps[:], bq_sb[:, bcol:bcol + 1])

            # The whole slot is staged PARTITION-MAJOR [128, 6192]: cols
            # 0..3071 = k (6 d-tiles x 512), cols 3072..6191 = v (4 chunks
            # x 780). One readback DMA per remote rank recovers everything.
            ag2 = ag_in.rearrange("(p x) -> p x", p=P)
            agk2 = ag2[:, 0:NDT * TL]
            agv2 = ag2[:, NDT * TL:SLOT // P]

            # k first (AllGather input): bias cols 6..11 of bq_sb.
            for i in range(NDT):
                proj_jt(wk_sb, i, NDT + i, kT_loc[i][:])
                nc.sync.dma_start(agk2[:, TL * i:TL * (i + 1)], kT_loc[i][:])
            for tt in range(NTT):
                vfv = v_loc[tt][:].rearrange("p (h e) -> p h e", e=HD + 1)
                nc.vector.memset(vfv[:, :, HD:HD + 1], 1.0)
                for o2 in range(2):
                    sl = slice(384 * o2, 384 * (o2 + 1))
                    ps = psV.tile([P, 384], F32, tag="psv", name="psv")
                    for d_ in range(NDT):
                        nc.tensor.matmul(ps[:],
                                         xtb_sb[:, d_, P * tt:P * (tt + 1)],
                                         wv_sb[:, d_, sl],
                                         start=(d_ == 0), stop=False)
                    nc.tensor.matmul(ps[:], ones_row[:], bv_sb[:, sl],
                                     start=False, stop=True)
                    dst = vfv[:, 6 * o2:6 * (o2 + 1), 0:HD]
                    nc.vector.tensor_copy(
                        dst, ps[:].rearrange("p (h e) -> p h e", e=HD))
                nc.sync.dma_start(agv2[:, VW * tt:VW * (tt + 1)], v_loc[tt][:])

            nc.gpsimd.collective_compute(
                "AllGather", mybir.AluOpType.bypass,
                replica_groups=[list(range(R))],
                ins=[ag_in[:]], outs=[ag_out[:]],
            )

            # q projections overlap with the collective (wq DMA deferred so
            # it never competes with the k/v path that gates the AllGather)
            nc.scalar.dma_start(wq_sb[:], wqc.ap())
            for i in range(NDT):
                proj_jt(wq_sb, i, i, qT[i][:])

        # ------------------------------------------------------------------
        # Phase A: local (diagonal) attention, fully overlapped with the
        # k/v AllGather (needs no remote data). Own pool scope so the local
        # k/v staging tiles can be released before the remote readback.
        # ------------------------------------------------------------------
        with ExitStack() as phA:
            scpA = phA.enter_context(tc.tile_pool(name="scpA", bufs=2,
                                                  space="PSUM"))
            accpA = phA.enter_context(tc.tile_pool(name="accpA", bufs=3,
                                                   space="PSUM"))
            weipA = phA.enter_context(tc.tile_pool(name="weipA", bufs=4))
            for p_ in range(H // 2):
                h0, h1 = 2 * p_, 2 * p_ + 1
                accA0 = accpA.tile([HD + 1, TL], F32, tag="acc", name="accA0")
                accA1 = accpA.tile([HD + 1, TL], F32, tag="acc", name="accA1")
                for s in range(NTT):
                    kT_c = kT_loc[p_][:, P * s:P * (s + 1)]
                    sc = scpA.tile([P, 2 * TL], F32, tag="sc", name="scA")
                    nc.tensor.matmul(sc[:, 0:TL], kT_c[0:HD, :],
                                     qT[p_][0:HD, :], start=True, stop=True)
                    nc.tensor.matmul(sc[:, TL:2 * TL], kT_c[HD:P, :],
                                     qT[p_][HD:P, :], start=True, stop=True)
                    wei = weipA.tile([P, 2 * TL], BF16, tag="wei", name="weiA")
                    nc.scalar.activation(wei[:], sc[:], AFT.Exp, scale=SCALE)
                    nc.tensor.matmul(
                        accA0[:], v_loc[s][:, (HD + 1) * h0:(HD + 1) * (h0 + 1)],
                        wei[:, 0:TL], start=(s == 0), stop=(s == NTT - 1))
                    nc.tensor.matmul(
                        accA1[:], v_loc[s][:, (HD + 1) * h1:(HD + 1) * (h1 + 1)],
                        wei[:, TL:2 * TL], start=(s == 0), stop=(s == NTT - 1))
                nc.vector.tensor_copy(locn[p_][0][:], accA0[:])
                nc.vector.tensor_copy(locn[p_][1][:], accA1[:])
        akv_scope.close()

        # ---- Remote k/v readback: one 2D dynamic-offset DMA per remote
        # rank per tensor (the DRAM offset is a register computed from
        # the partition id, so the local rank is skipped). Interleaved
        # k0,v0,k1,v1,... in consumption order.
        rkvp = kv_scope.enter_context(tc.tile_pool(name="rkvp", bufs=1))
        kvr = [rkvp.tile([P, SLOT // P], BF16, name=f"kvr{j}")
               for j in range(R - 1)]
        rank_g = nc.gpsimd.partition_id()
        for j in range(R - 1):
            off = ((rank_g + (1 + j)) % R) * SLOT
            full = ag_out[bass.ds(off, SLOT)].rearrange("(p x) -> p x", p=P)
            if j == 0:
                # split the first rank's readback so pair-0 scores only
                # wait for the (smaller) k half
                nc.gpsimd.dma_start(kvr[0][:, 0:NDT * TL],
                                    full[:, 0:NDT * TL])
                nc.gpsimd.dma_start(kvr[0][:, NDT * TL:SLOT // P],
                                    full[:, NDT * TL:SLOT // P])
            else:
                nc.gpsimd.dma_start(kvr[j][:], full)

        def kT_of(j, p_, s):
            return kvr[j][:, TL * p_ + P * s:TL * p_ + P * (s + 1)]

        def v_of(j, s, h):
            base = NDT * TL + VW * s + (HD + 1) * h
            return kvr[j][:, base:base + HD + 1]

        # FFN weight prefetch on the sync ring, poisoned on the last
        # readback so it doesn't compete with the collective for HBM
        nc.vector.tensor_copy(w1sb[0:1, 0, 0:1], kvr[R - 2][0:1, 0:1])
        nc.sync.dma_start(w1sb[:], w1h.ap())
        nc.vector.tensor_copy(w2sb[0:1, 0, 0:1], kvr[R - 2][0:1, 0:1])
        nc.sync.dma_start(w2sb[:], w2b.ap())

        # ------------------------------------------------------------------
        # Phase B: attention over the 28 remote chunks per head pair
        # ------------------------------------------------------------------
        with ExitStack() as ph2:
            scp = ph2.enter_context(tc.tile_pool(name="scp", bufs=2, space="PSUM"))
            accp = ph2.enter_context(tc.tile_pool(name="accp", bufs=3, space="PSUM"))
            bcp = ph2.enter_context(tc.tile_pool(name="bcp", bufs=1, space="PSUM"))
            weip = ph2.enter_context(tc.tile_pool(name="weip", bufs=4))
            tailp = ph2.enter_context(tc.tile_pool(name="tailp", bufs=2))

            def finish_pair(pend):
                """Normalize a finished pair: ONE merged reciprocal [2,TL]
                (halves the slow DVE recip cost) -> ONE sel2 broadcast matmul
                (K=2 picks rec row 0 for head-0 partitions, row 1 for head-1)
                -> multiply into attnT straight from PSUM. Emitted early in
                the NEXT pair so nothing lands on a pair boundary."""
                pp, num, den2 = pend
                rec2 = tailp.tile([33, TL], F32R, tag="rec2", name="rec2")
                with nc.allow_low_precision(reason="f32r recip, as baseline"):
                    nc.vector.reciprocal(rec2[:], den2[:].bitcast(F32R))
                bc = bcp.tile([P, TL], F32, tag="bc", name="bc")
                nc.tensor.matmul(bc[:], sel33[:, 0:P], rec2[:],
                                 start=True, stop=True)
                nc.vector.tensor_tensor(attnT[pp][:], num[:].bitcast(F32R),
                                        bc[:].bitcast(F32R),
                                        mybir.AluOpType.mult)

            # Phase B runs as ONE software-pipelined stream over all
            # (pair, chunk): the score/exp stream never drains at a pair
            # boundary; the wei@v consumer trails by two chunks, seeding
            # each pair's accumulators (phase-A partials, identity matmul)
            # right before its first wei@v and evacuating right after its
            # last.
            pend = [None]
            wq_ = []
            accs = {}

            den_pre = {}

            def emit_weiv():
                pw, cq, wei = wq_.pop(0)
                hh0, hh1 = 2 * pw, 2 * pw + 1
                if cq == 16:
                    # pre-allocate + memset the denominator tile mid-pair so
                    # only the two row copies remain on the finish chain
                    dp = tailp.tile([33, TL], F32, tag="den2", name="den2")
                    nc.vector.memset(dp[:], 1.0)
                    den_pre[pw] = dp
                if cq == 0:
                    a0 = accp.tile([HD + 1, TL], F32, tag="acc", name="acc0")
                    a1 = accp.tile([HD + 1, TL], F32, tag="acc", name="acc1")
                    accs[pw] = (a0, a1)
                    nc.tensor.matmul(a0[:], i65[:, 0:HD + 1],
                                     locn[pw][0][:], start=True, stop=False)
                    nc.tensor.matmul(a1[:], i65[:, 0:HD + 1],
                                     locn[pw][1][:], start=True, stop=False)
                a0, a1 = accs[pw]
                j, s = cq // NTT, cq % NTT
                stop = cq == NRC - 1
                nc.tensor.matmul(a0[:], v_of(j, s, hh0),
                                 wei[:, 0:TL], start=False, stop=stop)
                nc.tensor.matmul(a1[:], v_of(j, s, hh1),
                                 wei[:, TL:2 * TL], start=False, stop=stop)
                if stop:
                    # Evacuate the denominators FIRST (the slow reciprocal
                    # is the long pole of the finish chain and the final
                    # pair's chain gates the FFN start), then numerators.
                    den2 = den_pre.pop(pw)
                    nc.vector.tensor_copy(den2[0:1, :], a0[HD:HD + 1, :])
                    nc.vector.tensor_copy(den2[32:33, :], a1[HD:HD + 1, :])
                    num = tailp.tile([P, TL], F32, tag="num", name="num")
                    nc.vector.tensor_copy(num[0:HD, :], a0[0:HD, :])
                    nc.vector.tensor_copy(num[HD:P, :], a1[0:HD, :])
                    pend[0] = (pw, num, den2)

            for p_ in range(H // 2):
                for c in range(NRC):
                    j, s = c // NTT, c % NTT
                    kT_c = kT_of(j, p_, s)
                    sc = scp.tile([P, 2 * TL], F32, tag="sc", name="sc")
                    nc.tensor.matmul(sc[:, 0:TL], kT_c[0:HD, :],
                                     qT[p_][0:HD, :], start=True, stop=True)
                    nc.tensor.matmul(sc[:, TL:2 * TL], kT_c[HD:P, :],
                                     qT[p_][HD:P, :], start=True, stop=True)
                    wei = weip.tile([P, 2 * TL], BF16, tag="wei", name="wei")
                    nc.scalar.activation(wei[:], sc[:], AFT.Exp, scale=SCALE)
                    wq_.append((p_, c, wei))
                    if len(wq_) >= 3:
                        emit_weiv()
                    if c == 8 and pend[0] is not None:
                        finish_pair(pend[0])
                        pend[0] = None
            while wq_:
                emit_weiv()
            finish_pair(pend[0])

        kv_scope.close()

        # ------------------------------------------------------------------
        # Phase 3: FFN1 (gelu) pipelined with FFN2 sweep A, then sweep B
        # ------------------------------------------------------------------
        hTp = top.enter_context(tc.tile_pool(name="hTp", bufs=1))
        hT = [hTp.tile([P, TL], BF16, name=f"hT{f}") for f in range(NFT)]

        with ExitStack() as ph3:
            ps1 = ph3.enter_context(tc.tile_pool(name="ps1", bufs=2, space="PSUM"))
            ps2 = ph3.enter_context(tc.tile_pool(name="ps2", bufs=1, space="PSUM"))
            outp = ph3.enter_context(tc.tile_pool(name="outp", bufs=1))

            # FFN2 computed transposed: outT[d, t] = sum_ff w2[ff, d] h[t, ff]
            # = one clean N=512 accumulation per d-block over all 24 ff-tiles.
            # Six [P, TL] accumulators (6 banks) + ps1 (2) fill PSUM exactly;
            # there is no separate tail sweep and the bias is a per-partition
            # add during evacuation.
            accO = [ps2.tile([P, TL], F32, tag=f"o{j}", name=f"accO{j}")
                    for j in range(NDT)]
            outT_sb = [outp.tile([P, TL], F32, name=f"outT{j}")
                       for j in range(NDT)]

            # The first two FFN1 chains emit their first 5 d-tiles before
            # either consumes attnT[5], so they stream under the final
            # pair's normalization chain instead of head-of-line blocking.
            ps_pre = []
            for ft in range(2):
                ps = ps1.tile([P, TL], F32, tag="ps1t", name="ps1t")
                for d_ in range(NDT - 1):
                    nc.tensor.matmul(ps[:], w1sb[:, d_, P * ft:P * (ft + 1)],
                                     attnT[d_][:],
                                     start=(d_ == 0), stop=False)
                ps_pre.append(ps)

            def ffn_tail(ft, ps):
                nc.scalar.activation(hT[ft][:], ps[:], AFT.Gelu,
                                     bias=b1_sb[:, ft:ft + 1])
                for j in range(NDT):
                    nc.tensor.matmul(accO[j][:],
                                     w2sb[:, ft, P * j:P * (j + 1)],
                                     hT[ft][:],
                                     start=(ft == 0), stop=(ft == NFT - 1))

            for ft in range(2):
                nc.tensor.matmul(ps_pre[ft][:],
                                 w1sb[:, NDT - 1, P * ft:P * (ft + 1)],
                                 attnT[NDT - 1][:], start=False, stop=True)
                ffn_tail(ft, ps_pre[ft])
            for ft in range(2, NFT):
                ps = ps1.tile([P, TL], F32, tag="ps1t", name="ps1t")
                for d_ in range(NDT):
                    nc.tensor.matmul(ps[:], w1sb[:, d_, P * ft:P * (ft + 1)],
                                     attnT[d_][:],
                                     start=(d_ == 0), stop=(d_ == NDT - 1))
                ffn_tail(ft, ps)
            for j in range(NDT):
                # split the evacuation across DVE and the (now idle) ACT so
                # the output tail isn't serialized on one engine
                if j % 2 == 0:
                    nc.vector.tensor_scalar_add(outT_sb[j][:], accO[j][:],
                                                b2t_sb[:, j:j + 1])
                else:
                    nc.scalar.activation(outT_sb[j][:], accO[j][:],
                                         AFT.Identity,
                                         bias=b2t_sb[:, j:j + 1])
                nc.sync.dma_start(y.ap()[P * j:P * (j + 1), :], outT_sb[j][:])

    return nc


def _get_nc():
    if "nc" not in _NC_CACHE:
        _NC_CACHE["nc"] = _build_nc()
    return _NC_CACHE["nc"]


def _prep_common(inputs):
    w_qkv = np.ascontiguousarray(np.asarray(inputs["w_qkv"], np.float32))
    w_ff1 = np.ascontiguousarray(np.asarray(inputs["w_ff1"], np.float32))
    common = {
        # q columns, bf16, d-tile-major: [128, 6, 768]
        "wqc": np.ascontiguousarray(
            w_qkv[:, 0:D].reshape(NDT, P, D).transpose(1, 0, 2)
        ).astype(ml_dtypes.bfloat16),
        # k columns, bf16, d-tile-major: [128, 6, 768]
        "wkc": np.ascontiguousarray(
            w_qkv[:, D:2 * D].reshape(NDT, P, D).transpose(1, 0, 2)
        ).astype(ml_dtypes.bfloat16),
        # v columns, bf16, d-tile-major: [128, 6, 768]
        "wv": np.ascontiguousarray(
            w_qkv[:, 2 * D:].reshape(NDT, P, D).transpose(1, 0, 2)
        ).astype(ml_dtypes.bfloat16),
        # w_ff1 d-tile-major, bf16: [128, 6, 3072]
        "w1h": np.ascontiguousarray(
            w_ff1.reshape(NDT, P, DFF).transpose(1, 0, 2)
        ).astype(ml_dtypes.bfloat16),
        # w_ff2 ff-tile-major, bf16: [128, 24, 768]
        "w2b": np.ascontiguousarray(
            np.asarray(inputs["w_ff2"], np.float32)
            .reshape(NFT, P, D).transpose(1, 0, 2)).astype(ml_dtypes.bfloat16),
        "b_qkv": np.ascontiguousarray(np.asarray(inputs["b_qkv"], np.float32)),
        "b_ff1": np.ascontiguousarray(np.asarray(inputs["b_ff1"], np.float32)),
        "b_ff2": np.ascontiguousarray(np.asarray(inputs["b_ff2"], np.float32)),
    }
    return common


def run_sharded(inputs, **run_kwargs):
    """Run the SPMD kernel; returns (full_output [1,4096,768], BassKernelResults)."""
    x = np.ascontiguousarray(np.asarray(inputs["x"], dtype=np.float32))
    assert x.shape == (1, T, D), x.shape
    common = _prep_common(inputs)
    in_maps = []
    for r in range(R):
        m = dict(common)
        xr = x[0, TL * r:TL * (r + 1), :]  # [512, 768]
        xT = np.ascontiguousarray(xr.T.reshape(NDT, P, TL).transpose(1, 0, 2))
        m["xTb"] = xT.astype(ml_dtypes.bfloat16)
        in_maps.append(m)
    nc = _get_nc()
    res = run_bass_kernel_spmd(nc, in_maps, core_ids=list(range(R)), **run_kwargs)
    out = np.concatenate([np.asarray(res.results[r]["y"]).T for r in range(R)],
                         axis=0)
    return np.ascontiguousarray(out).reshape(1, T, D), res


def kernel(**inputs):
    out, _ = run_sharded(inputs)
    return out

